# revision 9
# baseline (speedup 1.0000x reference)
"""Trainium2 Bass kernel for nn_Attention_msa (sparse cosine attention).

Head-sharded across 8 NeuronCores: core h computes head h of
  qkv = x @ W^T;  qn,kn,vn cosine-normalized
  attn = softmax((qn@kn^T) * 25 * cs[j] * mask[i,j])   mask = cs[j] > cs[i]-0.1
  x_h = attn @ v  ;  x_ori_h = v
  vv = vn@vn^T
The head-summed attn and vv go through a chunked fp16 ReduceScatter; each core
then finishes sim_round2 = renorm(mask(sim_raw>0.75) * softmax(mean_h attn))
for its slice of rows.

Layout notes:
  - Everything on-chip is kept in "transposed" [d, n] layout so the d=128
    head dim is the partition/contraction axis.
  - Host pre-transposes x (to x^T [C, N]) and the per-head W slices.
  - Outputs: out_xT [128, N] = (attn@v)^T, out_vT [128, N] = v^T,
    out_sim [N/8, N]. Host transposes/concats.
"""

import sys
import numpy as np

for _p in ("/opt/trn_rl_repo", "/root/.axon_site/_ro/trn_rl_repo"):
    if _p not in sys.path:
        sys.path.insert(0, _p)

import concourse.bass as bass
import concourse.tile as tile
from concourse import bacc, mybir
from concourse.masks import make_identity

F32 = mybir.dt.float32
F32R = mybir.dt.float32r
F16 = mybir.dt.float16
AF = mybir.ActivationFunctionType
ALU = mybir.AluOpType

N = 2048
C = 1024
H = 8
HD = 128
SCALE = 25.0
SIM_THRESH = 0.75
N_CORES = 8
N_BLOCKS = N // 128          # 16 i-blocks of 128 rows
N_RS = 4                     # number of chunked ReduceScatter calls
BG = N_BLOCKS // N_RS        # i-blocks per RS group
S_ROWS = 128 * BG // N_CORES  # rows per RS slot (64)
KC = C // 128                # contraction chunks for projections (8)
NF = N // 512                # 512-wide free-dim chunks (4)


def build_nc(fp32r_matmul=True):
    nc = bacc.Bacc("TRN2", target_bir_lowering=False, debug=False,
                   num_devices=N_CORES)

    # Kernel I/O (per-core; SPMD identical program, different data)
    xT_d = nc.dram_tensor("xT", [C, N], F32R, kind="ExternalInput")
    wT_d = nc.dram_tensor("wT", [C, 3 * HD], F32R, kind="ExternalInput")
    cs_d = nc.dram_tensor("cs", [N], F32, kind="ExternalInput")
    out_xT = nc.dram_tensor("out_xT", [HD, N], F32, kind="ExternalOutput")
    out_vT = nc.dram_tensor("out_vT", [HD, N], F32, kind="ExternalOutput")
    out_sim = nc.dram_tensor("out_sim", [N // N_CORES, N], F32,
                             kind="ExternalOutput")

    HN = N // 2  # half-width S-psum chunk

    from contextlib import ExitStack
    with tile.TileContext(nc) as tc, ExitStack() as ctx:
        singles = ctx.enter_context(tc.tile_pool(name="singles", bufs=1))
        io_ctx = tc.tile_pool(name="io", bufs=1)
        io = io_ctx.__enter__()
        ps_s = ctx.enter_context(tc.tile_pool(name="ps_s", bufs=2, space="PSUM"))
        ps_tp = ctx.enter_context(tc.tile_pool(name="ps_tp", bufs=2, space="PSUM"))
        ps_pv = ctx.enter_context(tc.tile_pool(name="ps_pv", bufs=2, space="PSUM"))
        dram = ctx.enter_context(tc.tile_pool(name="dram", bufs=1, space="DRAM"))

        # ---- constants / inputs (xT chunked per-kc so proj starts early) ----
        xT = io.tile([128, KC, N], F32R, tag="xT")
        xTr = xT_d.ap().rearrange("(k p) n -> p k n", p=128)
        for kc in range(KC):
            nc.sync.dma_start(out=xT[:, kc, :], in_=xTr[:, kc, :])
        wT = io.tile([128, KC, 3 * HD], F32R, tag="wT")
        nc.sync.dma_start(out=wT, in_=wT_d.ap().rearrange("(k p) m -> p k m", p=128))

        cs_b = singles.tile([128, N], F32, tag="cs_b")
        nc.sync.dma_start(out=cs_b, in_=bass.AP(
            tensor=cs_d, offset=0, ap=[[0, 128], [1, N]]))
        # cs gathered per-partition: [128, N_BLOCKS] where col b = cs[128b + p]
        cs_col = singles.tile([128, N_BLOCKS], F32, tag="cs_col")
        nc.sync.dma_start(out=cs_col, in_=bass.AP(
            tensor=cs_d, offset=0, ap=[[1, 128], [128, N_BLOCKS]]))
        csm01 = singles.tile([128, N_BLOCKS], F32, tag="csm01")
        nc.vector.tensor_scalar_add(csm01, cs_col, -0.1)

        ident16 = singles.tile([128, 128], F16, tag="ident16")
        make_identity(nc, ident16)
        ones = singles.tile([128, 1], F32, tag="ones")
        nc.vector.memset(ones, 1.0)

        # ---- projections: tT[d, n] = sum_c wT[c, d] * xT[c, n] ----
        proj = {}
        for tname in ("v", "k", "q"):
            tcol = {"q": 0, "k": 1, "v": 2}[tname]
            t_sb = singles.tile([128, N], F32R, tag=f"t_{tname}")
            for hc in range(2):
                psum = ps_s.tile([128, HN], F32, tag="s", name=f"ps_{tname}{hc}")
                for kc in range(KC):
                    lhsT = wT[:, kc, tcol * HD:(tcol + 1) * HD]
                    for nf in range(HN // 512):
                        o = nf * 512
                        nc.tensor.matmul(
                            psum[:, o:o + 512], lhsT,
                            xT[:, kc, hc * HN + o:hc * HN + o + 512],
                            start=(kc == 0), stop=(kc == KC - 1))
                nc.scalar.copy(t_sb[:, hc * HN:(hc + 1) * HN], psum)
            proj[tname] = t_sb
        io_ctx.__exit__(None, None, None)
        rows = ctx.enter_context(tc.tile_pool(name="rows", bufs=4))
        work = ctx.enter_context(tc.tile_pool(name="work", bufs=2))

        # ---- inverse norms along d (partition) via ones-matmul;
        #      rows go sumsq -> sqrt -> reciprocal in place ----
        inv = {}
        for tname in ("v", "k", "q"):
            sq = work.tile([128, N], F32, tag="sq")
            nc.vector.tensor_mul(sq, proj[tname], proj[tname])
            ss_parts = []
            for nf in range(NF):
                ssp = ps_tp.tile([1, 512], F32, tag="tp", name=f"ssp_{tname}{nf}")
                nc.tensor.matmul(ssp, ones, sq[:, nf * 512:(nf + 1) * 512],
                                 start=True, stop=True)
                ss_parts.append(ssp)
            irow = rows.tile([1, N], F32, tag="row", name=f"inv_{tname}")
            for nf in range(NF):
                nc.vector.tensor_copy(irow[:, nf * 512:(nf + 1) * 512],
                                      ss_parts[nf])
            nc.scalar.sqrt(irow, irow)
            nc.vector.reciprocal(irow, irow)
            inv[tname] = irow

        # colfac[j] = SCALE * cs[j] * inv_k[j]
        colfac = rows.tile([1, N], F32, tag="row", name="colfac")
        nc.vector.tensor_scalar_mul(colfac, inv["k"], SCALE)
        nc.vector.tensor_mul(colfac, colfac, cs_b[0:1, :])

        # broadcast colfac / inv_v / inv_q to all partitions via DRAM bounce
        bcast = {}
        for nm, row in (("colfac", colfac), ("invv", inv["v"]),
                        ("invq", inv["q"])):
            dsc = dram.tile([N], F32, tag=f"dsc_{nm}", name=f"dsc_{nm}")
            nc.sync.dma_start(out=dsc, in_=row)
            bt = singles.tile([128, N], F32, tag=f"b_{nm}", name=f"b_{nm}")
            nc.sync.dma_start(out=bt, in_=bass.AP(
                tensor=dsc.tensor, offset=dsc.offset, ap=[[0, 128], [1, N]]))
            bcast[nm] = bt

        # out_vT is the raw projected v (exact x_ori)
        nc.sync.dma_start(out=out_vT.ap(), in_=proj["v"].bitcast(F32))

        # raw v in fp16, natural layout [n, d] for the PV matmul
        vT16 = singles.tile([128, N], F16, tag="vT16")
        nc.vector.tensor_copy(vT16, proj["v"])
        v16n = singles.tile([128, N_BLOCKS, 128], F16, tag="v16n")
        for b in range(N_BLOCKS):
            tp = ps_tp.tile([128, 128], F16, tag="tp", name=f"vtp{b}")
            nc.tensor.transpose(tp, vT16[:, b * 128:(b + 1) * 128], ident16)
            nc.vector.tensor_copy(v16n[:, b, :], tp)

        # column scales written in place (bitcast to f32r so walrus sees
        # TF32-rounded matmul operands); Tile orders them after all raw reads
        knT = proj["k"]
        nc.vector.tensor_mul(knT, proj["k"], bcast["colfac"])
        vnT = proj["v"]
        nc.vector.tensor_mul(vnT, proj["v"], bcast["invv"])
        qT = proj["q"]
        nc.vector.tensor_mul(qT, proj["q"], bcast["invq"])

        # ---- ReduceScatter staging: per 8-block group, one [8,128,N] buffer
        #      for attn and one for vv; slot c = block 8k+c ----
        NG = N_BLOCKS // 8  # groups (2)
        rs_in_a = [dram.tile([8, 128, N], F16, tag=f"rs_in_a{k}",
                             name=f"rs_in_a{k}") for k in range(NG)]
        rs_in_v = [dram.tile([8, 128, N], F16, tag=f"rs_in_v{k}",
                             name=f"rs_in_v{k}") for k in range(NG)]
        rs_out_a = [dram.tile([128, N], F16, tag=f"rs_out_a{k}",
                              name=f"rs_out_a{k}") for k in range(NG)]
        rs_out_v = [dram.tile([128, N], F16, tag=f"rs_out_v{k}",
                              name=f"rs_out_v{k}") for k in range(NG)]

        # ---- VV phase: vv = vnT.T @ vnT (fully normalized) ----
        for b in range(N_BLOCKS):
            vv16 = work.tile([128, N], F16, tag="vv16")
            for hc in range(2):
                psum = ps_s.tile([128, HN], F32, tag="s", name=f"vvps{b}_{hc}")
                for nf in range(HN // 512):
                    o = nf * 512
                    nc.tensor.matmul(psum[:, o:o + 512],
                                     vnT[:, b * 128:(b + 1) * 128],
                                     vnT[:, hc * HN + o:hc * HN + o + 512],
                                     start=True, stop=True)
                if hc == 0:
                    nc.scalar.copy(vv16[:, 0:HN], psum)
                else:
                    nc.vector.tensor_copy(vv16[:, HN:N], psum)
            nc.sync.dma_start(out=rs_in_v[b // 8][b % 8], in_=vv16)
        for k in range(NG):
            nc.gpsimd.collective_compute(
                "ReduceScatter", ALU.add,
                replica_groups=[list(range(N_CORES))],
                ins=[rs_in_v[k].opt()], outs=[rs_out_v[k].opt()])

        # ---- QK/softmax/PV phase ----
        for b in range(N_BLOCKS):
            xm = work.tile([128, N], F32, tag="xm")
            mask = work.tile([128, N], F32, tag="mask")
            nc.gpsimd.tensor_scalar(mask, cs_b, csm01[:, b:b + 1], None,
                                    op0=ALU.is_gt)
            for hc in range(2):
                psum = ps_s.tile([128, HN], F32, tag="s", name=f"qkps{b}_{hc}")
                for nf in range(HN // 512):
                    o = nf * 512
                    nc.tensor.matmul(psum[:, o:o + 512],
                                     qT[:, b * 128:(b + 1) * 128],
                                     knT[:, hc * HN + o:hc * HN + o + 512],
                                     start=True, stop=True)
                nc.vector.tensor_tensor(xm[:, hc * HN:(hc + 1) * HN], psum,
                                        mask[:, hc * HN:(hc + 1) * HN],
                                        op=ALU.mult)
            attn_un = work.tile([128, N], F32, tag="attn_un")
            rowsum = work.tile([128, 1], F32, tag="rowsum")
            nc.scalar.activation(attn_un, xm, AF.Exp, accum_out=rowsum)
            inv_rs = work.tile([128, 1], F32, tag="inv_rs")
            nc.vector.reciprocal(inv_rs, rowsum)
            attn16 = work.tile([128, N], F16, tag="attn16")
            nc.gpsimd.tensor_scalar(attn16, attn_un, inv_rs, None, op0=ALU.mult)
            nc.sync.dma_start(out=rs_in_a[b // 8][b % 8], in_=attn16)
            # PV: xT_blk[d, i] accumulated over j-tiles of attn^T
            attnT = work.tile([128, N_BLOCKS, 128], F16, tag="attnT")
            for j in range(N_BLOCKS):
                tp = ps_tp.tile([128, 128], F16, tag="tp", name=f"atp{b}_{j}")
                nc.tensor.transpose(tp, attn16[:, j * 128:(j + 1) * 128],
                                    ident16)
                nc.vector.tensor_copy(attnT[:, j, :], tp)
            pv = ps_pv.tile([128, 128], F32, tag="pv")
            for j in range(N_BLOCKS):
                nc.tensor.matmul(pv, v16n[:, j, :], attnT[:, j, :],
                                 start=(j == 0), stop=(j == N_BLOCKS - 1))
            xout = work.tile([128, 128], F32, tag="xout")
            nc.vector.tensor_copy(xout, pv)
            nc.sync.dma_start(out=out_xT.ap()[:, b * 128:(b + 1) * 128],
                              in_=xout)
            if b % 8 == 7:
                nc.gpsimd.collective_compute(
                    "ReduceScatter", ALU.add,
                    replica_groups=[list(range(N_CORES))],
                    ins=[rs_in_a[b // 8].opt()], outs=[rs_out_a[b // 8].opt()])

        # ---- final sim chain per group: rows 1024k + 128*core ----
        for k in range(NG):
            ta = work.tile([128, N], F16, tag="attn16", name=f"fin_a{k}")
            nc.sync.dma_start(out=ta, in_=rs_out_a[k])
            tv = work.tile([128, N], F16, tag="vv16", name=f"fin_v{k}")
            nc.sync.dma_start(out=tv, in_=rs_out_v[k])
            e = work.tile([128, N], F32, tag="xm", name=f"fin_e{k}")
            nc.scalar.activation(e, ta, AF.Exp, scale=1.0 / H)
            m2 = work.tile([128, N], F32, tag="mask", name=f"fin_m{k}")
            nc.vector.tensor_scalar(m2, tv, float(SIM_THRESH * H), None,
                                    op0=ALU.is_gt)
            nc.vector.tensor_mul(e, e, m2)
            msum = work.tile([128, 1], F32, tag="rowsum", name=f"fin_ms{k}")
            nc.vector.reduce_sum(msum, e, axis=mybir.AxisListType.X)
            minv = work.tile([128, 1], F32, tag="inv_rs", name=f"fin_mi{k}")
            nc.vector.reciprocal(minv, msum)
            outt = work.tile([128, N], F32, tag="attn_un", name=f"fin_out{k}")
            nc.vector.tensor_scalar(outt, e, minv, None, op0=ALU.mult)
            nc.sync.dma_start(
                out=out_sim.ap()[k * 128:(k + 1) * 128, :], in_=outt)

    nc.compile()
    return nc


_NC_CACHE = {}


def tf32_round(a):
    u = np.ascontiguousarray(a, dtype=np.float32).view(np.uint32)
    r = (u + np.uint32(0x1000) + ((u >> np.uint32(13)) & np.uint32(1))) \
        & ~np.uint32(0x1FFF)
    return r.view(np.float32)


def kernel(x_cls, cls_score, fg_score, W_qkv):
    from concourse.bass_utils import run_bass_kernel_spmd

    x_cls = np.asarray(x_cls)
    cls_score = np.asarray(cls_score, dtype=np.float32)
    W_qkv = np.asarray(W_qkv, dtype=np.float32)
    B = x_cls.shape[0]
    xT = tf32_round(np.ascontiguousarray(x_cls.reshape(N, C).T.astype(np.float32)))

    if "nc" not in _NC_CACHE:
        _NC_CACHE["nc"] = build_nc()
    nc = _NC_CACHE["nc"]

    in_maps = []
    for h in range(N_CORES):
        w_h = np.concatenate([
            W_qkv[0 * C + h * HD:0 * C + (h + 1) * HD],   # q rows [HD, C]
            W_qkv[1 * C + h * HD:1 * C + (h + 1) * HD],   # k rows
            W_qkv[2 * C + h * HD:2 * C + (h + 1) * HD],   # v rows
        ], axis=0)                                        # [3HD, C]
        wT_h = tf32_round(np.ascontiguousarray(w_h.T))    # [C, 3HD]
        in_maps.append({"xT": xT, "wT": wT_h, "cs": cls_score})

    res = run_bass_kernel_spmd(nc, in_maps, list(range(N_CORES)))
    outs = res.results

    x = np.empty((N, C), np.float32)
    x_ori = np.empty((N, C), np.float32)
    sim = np.empty((N, N), np.float32)
    for h in range(N_CORES):
        x[:, h * HD:(h + 1) * HD] = outs[h]["out_xT"].T
        x_ori[:, h * HD:(h + 1) * HD] = outs[h]["out_vT"].T
        os = outs[h]["out_sim"]                           # [2*128, N]
        for k in range(N // (8 * 128)):
            r0 = 1024 * k + 128 * h
            sim[r0:r0 + 128] = os[k * 128:(k + 1) * 128]
    x_out = np.concatenate([x, x_ori], axis=-1).reshape(B, N, 2 * C)
    return x_out, sim


# revision 10
# speedup vs baseline: 2.8875x; 2.8875x over previous
"""Trainium2 Bass kernel for nn_Attention_msa (sparse cosine attention).

Head-sharded across 8 NeuronCores: core h computes head h of
  qkv = x @ W^T;  qn,kn,vn cosine-normalized
  attn = softmax((qn@kn^T) * 25 * cs[j] * mask[i,j])   mask = cs[j] > cs[i]-0.1
  x_h = attn @ v  ;  x_ori_h = v
  vv = vn@vn^T
The head-summed attn and vv go through a chunked fp16 ReduceScatter; each core
then finishes sim_round2 = renorm(mask(sim_raw>0.75) * softmax(mean_h attn))
for its slice of rows.

Layout notes:
  - Everything on-chip is kept in "transposed" [d, n] layout so the d=128
    head dim is the partition/contraction axis.
  - Host pre-transposes x (to x^T [C, N]) and the per-head W slices.
  - Outputs: out_xT [128, N] = (attn@v)^T, out_vT [128, N] = v^T,
    out_sim [N/8, N]. Host transposes/concats.
"""

import sys
import numpy as np

for _p in ("/opt/trn_rl_repo", "/root/.axon_site/_ro/trn_rl_repo"):
    if _p not in sys.path:
        sys.path.insert(0, _p)

import concourse.bass as bass
import concourse.tile as tile
from concourse import bacc, mybir
from concourse.masks import make_identity

F32 = mybir.dt.float32
F32R = mybir.dt.float32r
F16 = mybir.dt.float16
AF = mybir.ActivationFunctionType
ALU = mybir.AluOpType

N = 2048
C = 1024
H = 8
HD = 128
SCALE = 25.0
SIM_THRESH = 0.75
N_CORES = 8
N_BLOCKS = N // 128          # 16 i-blocks of 128 rows
N_RS = 4                     # number of chunked ReduceScatter calls
BG = N_BLOCKS // N_RS        # i-blocks per RS group
S_ROWS = 128 * BG // N_CORES  # rows per RS slot (64)
KC = C // 128                # contraction chunks for projections (8)
NF = N // 512                # 512-wide free-dim chunks (4)


def build_nc(fp32r_matmul=True):
    nc = bacc.Bacc("TRN2", target_bir_lowering=False, debug=False,
                   num_devices=N_CORES)

    # Kernel I/O (per-core; SPMD identical program, different data)
    xT_d = nc.dram_tensor("xT", [C, N], F32R, kind="ExternalInput")
    wT_d = nc.dram_tensor("wT", [C, 3 * HD], F32R, kind="ExternalInput")
    cs_d = nc.dram_tensor("cs", [N], F32, kind="ExternalInput")
    out_xT = nc.dram_tensor("out_xT", [HD, N], F32, kind="ExternalOutput")
    out_vT = nc.dram_tensor("out_vT", [HD, N], F32, kind="ExternalOutput")
    out_sim = nc.dram_tensor("out_sim", [N // N_CORES, N], F32,
                             kind="ExternalOutput")

    HN = N // 2  # half-width S-psum chunk

    from contextlib import ExitStack
    with tile.TileContext(nc) as tc, ExitStack() as ctx:
        singles = ctx.enter_context(tc.tile_pool(name="singles", bufs=1))
        io_ctx = tc.tile_pool(name="io", bufs=1)
        io = io_ctx.__enter__()
        ps_s = ctx.enter_context(tc.tile_pool(name="ps_s", bufs=2, space="PSUM"))
        ps_tp = ctx.enter_context(tc.tile_pool(name="ps_tp", bufs=2, space="PSUM"))
        ps_pv = ctx.enter_context(tc.tile_pool(name="ps_pv", bufs=2, space="PSUM"))
        dram = ctx.enter_context(tc.tile_pool(name="dram", bufs=1, space="DRAM"))

        # ---- constants / inputs (xT chunked per-kc so proj starts early) ----
        wT = io.tile([128, KC, 3 * HD], F32R, tag="wT")
        nc.sync.dma_start(out=wT, in_=wT_d.ap().rearrange("(k p) m -> p k m", p=128))
        xT = io.tile([128, KC, N], F32R, tag="xT")
        xTr = xT_d.ap().rearrange("(k p) n -> p k n", p=128)
        for kc in range(KC):
            nc.sync.dma_start(out=xT[:, kc, :], in_=xTr[:, kc, :])

        cs_b = singles.tile([128, N], F32, tag="cs_b")
        nc.sync.dma_start(out=cs_b, in_=bass.AP(
            tensor=cs_d, offset=0, ap=[[0, 128], [1, N]]))
        # cs gathered per-partition: [128, N_BLOCKS] where col b = cs[128b + p]
        cs_col = singles.tile([128, N_BLOCKS], F32, tag="cs_col")
        nc.sync.dma_start(out=cs_col, in_=bass.AP(
            tensor=cs_d, offset=0, ap=[[1, 128], [128, N_BLOCKS]]))
        csm01 = singles.tile([128, N_BLOCKS], F32, tag="csm01")
        nc.vector.tensor_scalar_add(csm01, cs_col, -0.1)

        ident16 = singles.tile([128, 128], F16, tag="ident16")
        make_identity(nc, ident16)
        ones = singles.tile([128, 1], F32, tag="ones")
        nc.vector.memset(ones, 1.0)

        # ---- projections: tT[d, n] = sum_c wT[c, d] * xT[c, n] ----
        proj = {}
        for tname in ("v", "k", "q"):
            tcol = {"q": 0, "k": 1, "v": 2}[tname]
            t_sb = singles.tile([128, N], F32R, tag=f"t_{tname}")
            for hc in range(2):
                psum = ps_s.tile([128, HN], F32, tag="s", name=f"ps_{tname}{hc}")
                for kc in range(KC):
                    lhsT = wT[:, kc, tcol * HD:(tcol + 1) * HD]
                    for nf in range(HN // 512):
                        o = nf * 512
                        nc.tensor.matmul(
                            psum[:, o:o + 512], lhsT,
                            xT[:, kc, hc * HN + o:hc * HN + o + 512],
                            start=(kc == 0), stop=(kc == KC - 1))
                nc.scalar.copy(t_sb[:, hc * HN:(hc + 1) * HN], psum)
            proj[tname] = t_sb
        io_ctx.__exit__(None, None, None)
        rows = ctx.enter_context(tc.tile_pool(name="rows", bufs=4))
        work = ctx.enter_context(tc.tile_pool(name="work", bufs=2))

        # out_vT is the raw projected v (exact x_ori); fp16 copy + natural
        # layout transposes for PV. Done early so the in-place vnT scale
        # (WAR on t_v) doesn't stall the VV phase.
        nc.sync.dma_start(out=out_vT.ap(), in_=proj["v"].bitcast(F32))
        vT16 = singles.tile([128, N], F16, tag="vT16")
        nc.vector.tensor_copy(vT16, proj["v"])
        v16n = singles.tile([128, N_BLOCKS, 128], F16, tag="v16n")
        for b in range(N_BLOCKS):
            tp = ps_tp.tile([128, 128], F16, tag="tp", name=f"vtp{b}")
            nc.tensor.transpose(tp, vT16[:, b * 128:(b + 1) * 128], ident16)
            nc.vector.tensor_copy(v16n[:, b, :], tp)

        # ---- inverse norms along d (partition): ones-matmul sumsq ->
        #      broadcast the raw sumsq row to all partitions, THEN
        #      rsqrt on [128, N] tiles (parallel lanes; [1, N] engine ops
        #      are single-partition and pathologically slow) ----
        bcast = {}
        for tname in ("v", "k", "q"):
            sq = work.tile([128, N], F32, tag="sq")
            nc.vector.tensor_mul(sq, proj[tname], proj[tname])
            ss_parts = []
            for nf in range(NF):
                ssp = ps_tp.tile([1, 512], F32, tag="tp", name=f"ssp_{tname}{nf}")
                nc.tensor.matmul(ssp, ones, sq[:, nf * 512:(nf + 1) * 512],
                                 start=True, stop=True)
                ss_parts.append(ssp)
            ssrow = rows.tile([1, N], F32, tag="row", name=f"ss_{tname}")
            for nf in range(NF):
                nc.scalar.copy(ssrow[:, nf * 512:(nf + 1) * 512],
                               ss_parts[nf])
            dsc = dram.tile([N], F32, tag=f"dsc_{tname}", name=f"dsc_{tname}")
            nc.sync.dma_start(out=dsc, in_=ssrow)
            bt = singles.tile([128, N], F32, tag=f"b_{tname}", name=f"b_{tname}")
            nc.sync.dma_start(out=bt, in_=bass.AP(
                tensor=dsc.tensor, offset=dsc.offset, ap=[[0, 128], [1, N]]))
            nc.scalar.sqrt(bt, bt)
            nc.vector.reciprocal(bt, bt)
            bcast[tname] = bt
        # colfac[j] = cs[j] * inv_k[j]  (SCALE folded into the exp)
        nc.vector.tensor_mul(bcast["k"], bcast["k"], cs_b)

        # column scales written in place (bitcast to f32r so walrus sees
        # TF32-rounded matmul operands); Tile orders them after all raw reads
        knT = proj["k"]
        nc.vector.tensor_mul(knT, proj["k"], bcast["k"])
        vnT = proj["v"]
        nc.vector.tensor_mul(vnT, proj["v"], bcast["v"])
        qT = proj["q"]
        nc.vector.tensor_mul(qT, proj["q"], bcast["q"])

        # ---- ReduceScatter staging: per 8-block group, one [8,128,N] buffer
        #      for attn and one for vv; slot c = block 8k+c ----
        NG = N_BLOCKS // 8  # groups (2)
        rs_in_a = [dram.tile([8, 128, N], F16, tag=f"rs_in_a{k}",
                             name=f"rs_in_a{k}") for k in range(NG)]
        rs_in_v = [dram.tile([8, 128, N], F16, tag=f"rs_in_v{k}",
                             name=f"rs_in_v{k}") for k in range(NG)]
        rs_out_a = [dram.tile([128, N], F16, tag=f"rs_out_a{k}",
                              name=f"rs_out_a{k}") for k in range(NG)]
        rs_out_v = [dram.tile([128, N], F16, tag=f"rs_out_v{k}",
                              name=f"rs_out_v{k}") for k in range(NG)]

        # ---- VV phase: vv = vnT.T @ vnT (fully normalized) ----
        for b in range(N_BLOCKS):
            vv16 = work.tile([128, N], F16, tag="vv16")
            for hc in range(2):
                psum = ps_s.tile([128, HN], F32, tag="s", name=f"vvps{b}_{hc}")
                for nf in range(HN // 512):
                    o = nf * 512
                    nc.tensor.matmul(psum[:, o:o + 512],
                                     vnT[:, b * 128:(b + 1) * 128],
                                     vnT[:, hc * HN + o:hc * HN + o + 512],
                                     start=True, stop=True)
                if hc == 0:
                    nc.scalar.copy(vv16[:, 0:HN], psum)
                else:
                    nc.vector.tensor_copy(vv16[:, HN:N], psum)
            nc.sync.dma_start(out=rs_in_v[b // 8][b % 8], in_=vv16)
        for k in range(NG):
            nc.gpsimd.collective_compute(
                "ReduceScatter", ALU.add,
                replica_groups=[list(range(N_CORES))],
                ins=[rs_in_v[k].opt()], outs=[rs_out_v[k].opt()])

        # ---- QK/softmax/PV phase ----
        for b in range(N_BLOCKS):
            xm = work.tile([128, N], F32, tag="xm")
            mask = work.tile([128, N], F32, tag="mask")
            nc.vector.tensor_scalar(mask, cs_b, csm01[:, b:b + 1], None,
                                    op0=ALU.is_gt)
            for hc in range(2):
                psum = ps_s.tile([128, HN], F32, tag="s", name=f"qkps{b}_{hc}")
                for nf in range(HN // 512):
                    o = nf * 512
                    nc.tensor.matmul(psum[:, o:o + 512],
                                     qT[:, b * 128:(b + 1) * 128],
                                     knT[:, hc * HN + o:hc * HN + o + 512],
                                     start=True, stop=True)
                nc.vector.tensor_tensor(xm[:, hc * HN:(hc + 1) * HN], psum,
                                        mask[:, hc * HN:(hc + 1) * HN],
                                        op=ALU.mult)
            attn_un = work.tile([128, N], F32, tag="attn_un")
            rowsum = work.tile([128, 1], F32, tag="rowsum")
            nc.scalar.activation(attn_un, xm, AF.Exp, scale=float(SCALE),
                                 accum_out=rowsum)
            inv_rs = work.tile([128, 1], F32, tag="inv_rs")
            nc.vector.reciprocal(inv_rs, rowsum)
            attn16 = work.tile([128, N], F16, tag="attn16")
            nc.vector.tensor_scalar(attn16, attn_un, inv_rs, None, op0=ALU.mult)
            nc.sync.dma_start(out=rs_in_a[b // 8][b % 8], in_=attn16)
            # PV: xT_blk[d, i] accumulated over j-tiles of attn^T
            attnT = work.tile([128, N_BLOCKS, 128], F16, tag="attnT")
            for j in range(N_BLOCKS):
                tp = ps_tp.tile([128, 128], F16, tag="tp", name=f"atp{b}_{j}")
                nc.tensor.transpose(tp, attn16[:, j * 128:(j + 1) * 128],
                                    ident16)
                nc.vector.tensor_copy(attnT[:, j, :], tp)
            pv = ps_pv.tile([128, 128], F32, tag="pv")
            for j in range(N_BLOCKS):
                nc.tensor.matmul(pv, v16n[:, j, :], attnT[:, j, :],
                                 start=(j == 0), stop=(j == N_BLOCKS - 1))
            xout = work.tile([128, 128], F32, tag="xout")
            nc.vector.tensor_copy(xout, pv)
            nc.sync.dma_start(out=out_xT.ap()[:, b * 128:(b + 1) * 128],
                              in_=xout)
            if b % 8 == 7:
                nc.gpsimd.collective_compute(
                    "ReduceScatter", ALU.add,
                    replica_groups=[list(range(N_CORES))],
                    ins=[rs_in_a[b // 8].opt()], outs=[rs_out_a[b // 8].opt()])

        # ---- final sim chain per group: rows 1024k + 128*core ----
        for k in range(NG):
            ta = work.tile([128, N], F16, tag="attn16", name=f"fin_a{k}")
            nc.sync.dma_start(out=ta, in_=rs_out_a[k])
            tv = work.tile([128, N], F16, tag="vv16", name=f"fin_v{k}")
            nc.sync.dma_start(out=tv, in_=rs_out_v[k])
            e = work.tile([128, N], F32, tag="xm", name=f"fin_e{k}")
            nc.scalar.activation(e, ta, AF.Exp, scale=1.0 / H)
            m2 = work.tile([128, N], F32, tag="mask", name=f"fin_m{k}")
            nc.vector.tensor_scalar(m2, tv, float(SIM_THRESH * H), None,
                                    op0=ALU.is_gt)
            nc.vector.tensor_mul(e, e, m2)
            msum = work.tile([128, 1], F32, tag="rowsum", name=f"fin_ms{k}")
            nc.vector.reduce_sum(msum, e, axis=mybir.AxisListType.X)
            minv = work.tile([128, 1], F32, tag="inv_rs", name=f"fin_mi{k}")
            nc.vector.reciprocal(minv, msum)
            outt = work.tile([128, N], F32, tag="attn_un", name=f"fin_out{k}")
            nc.vector.tensor_scalar(outt, e, minv, None, op0=ALU.mult)
            nc.sync.dma_start(
                out=out_sim.ap()[k * 128:(k + 1) * 128, :], in_=outt)

    nc.compile()
    return nc


_NC_CACHE = {}


def tf32_round(a):
    u = np.ascontiguousarray(a, dtype=np.float32).view(np.uint32)
    r = (u + np.uint32(0x1000) + ((u >> np.uint32(13)) & np.uint32(1))) \
        & ~np.uint32(0x1FFF)
    return r.view(np.float32)


def kernel(x_cls, cls_score, fg_score, W_qkv):
    from concourse.bass_utils import run_bass_kernel_spmd

    x_cls = np.asarray(x_cls)
    cls_score = np.asarray(cls_score, dtype=np.float32)
    W_qkv = np.asarray(W_qkv, dtype=np.float32)
    B = x_cls.shape[0]
    xT = tf32_round(np.ascontiguousarray(x_cls.reshape(N, C).T.astype(np.float32)))

    if "nc" not in _NC_CACHE:
        _NC_CACHE["nc"] = build_nc()
    nc = _NC_CACHE["nc"]

    in_maps = []
    for h in range(N_CORES):
        w_h = np.concatenate([
            W_qkv[0 * C + h * HD:0 * C + (h + 1) * HD],   # q rows [HD, C]
            W_qkv[1 * C + h * HD:1 * C + (h + 1) * HD],   # k rows
            W_qkv[2 * C + h * HD:2 * C + (h + 1) * HD],   # v rows
        ], axis=0)                                        # [3HD, C]
        wT_h = tf32_round(np.ascontiguousarray(w_h.T))    # [C, 3HD]
        in_maps.append({"xT": xT, "wT": wT_h, "cs": cls_score})

    res = run_bass_kernel_spmd(nc, in_maps, list(range(N_CORES)))
    outs = res.results

    x = np.empty((N, C), np.float32)
    x_ori = np.empty((N, C), np.float32)
    sim = np.empty((N, N), np.float32)
    for h in range(N_CORES):
        x[:, h * HD:(h + 1) * HD] = outs[h]["out_xT"].T
        x_ori[:, h * HD:(h + 1) * HD] = outs[h]["out_vT"].T
        os = outs[h]["out_sim"]                           # [2*128, N]
        for k in range(N // (8 * 128)):
            r0 = 1024 * k + 128 * h
            sim[r0:r0 + 128] = os[k * 128:(k + 1) * 128]
    x_out = np.concatenate([x, x_ori], axis=-1).reshape(B, N, 2 * C)
    return x_out, sim
